# revision 20
# baseline (speedup 1.0000x reference)
"""Trainium2 Bass kernel for nn_Evaluation_78383153152424.

Sharding: 8 cores = 2 batches x 4 D-groups (8 planes each). Zero halo,
zero collectives: the 1x1x1 conv chain is pointwise in (d,h,w) and the
9-neighbor gather is local to each (b,d) HxW plane, which each core holds
in full (H=128 rows = 128 SBUF partitions).

Per-core pipeline (SBUF data fp16, PSUM fp32):
  conv chain as block-diagonal matmuls over 16 row-chunks (chunk = 8 rows
  x 160 cols), K=128 contraction; stage-3 col-tiled (tile_position) so a
  group of planes' sim rows share one PSUM tile -> single-op evacuation.
  The sim plane is stored to a reflect-padded DRAM scratch [136,160] per
  plane; ONE 3-dim DMA per plane then loads all five y-shift variants
  (rows y+dy, dy=2*slot-4) with reflected edges already in place. x-pads
  are reflected in with two DVE copies. The 9-neighbor two-dilation
  gather runs on DVE in fp16: terms are processed in PAIRS via one
  4-dim-AP tensor_tensor mul per pair plus pairwise-tree adds (26 ops
  per plane group instead of 35), x-shifts as free AP offsets, per-term
  weights pre-multiplied by 0.5*weight and replicated over d.
"""

import os
import sys
import functools

import numpy as np

for _p in ("/opt/trn_rl_repo", "/root/.axon_site/_ro/trn_rl_repo"):
    if os.path.isdir(_p) and _p not in sys.path:
        sys.path.append(_p)

import concourse.bass as bass
import concourse.tile as tile
from concourse import bacc, mybir
from concourse.bass_utils import run_bass_kernel_spmd

F16, F32 = mybir.dt.float16, mybir.dt.float32
AF = mybir.ActivationFunctionType
OP = mybir.AluOpType

B, G, D, H, W = 2, 8, 32, 128, 160
DG = 8                       # d-planes per core
NCHUNK, RPC = 16, 8          # chunks per plane, rows per chunk
CHUNK_F = RPC * W            # 1280 chunk-local positions
BLOCKS = [(0, 512), (512, 512), (1024, 256)]
XPAD = W + 8                 # 168: x-padded row
import os as _os
GROUPS = [int(c) for c in _os.environ.get("K_GROUPS", "2222")]
GMAX = max(GROUPS)
EVAC_DVE = _os.environ.get("K_EVAC_DVE", "1") == "1"
RELU1_DVE_PLANES = set()     # relu1 of these planes runs on DVE (balance)
WARMUP_MM = 45               # HAM warmup matmuls (~4.3us at N=128 cold)

# gather terms: (dy, dx, wall_slice). offset[:, s] weights the wide
# (stride-4) crop, offset[:, s+9] the narrow (stride-2) crop.
TERMS = []
for _s in range(9):
    _iy, _ix = _s // 3, _s % 3
    TERMS.append(((_iy - 1) * 2, (_ix - 1) * 2, 9 + _s))
    TERMS.append(((_iy - 1) * 4, (_ix - 1) * 4, _s))


def _ap(t, extra_off, dims):
    return bass.AP(tensor=t.tensor, offset=t.offset + extra_off, ap=dims)


@functools.lru_cache(maxsize=4)
def _build(zb0, zb1, zbsim):
    nc = bacc.Bacc("TRN2", target_bir_lowering=False, debug=False, num_devices=8)

    x_ap = nc.dram_tensor("x", [NCHUNK, G, DG * CHUNK_F], F16,
                          kind="ExternalInput").ap()
    wts_ap = nc.dram_tensor("wts", [128, 544], F16, kind="ExternalInput").ap()
    ow_ap = nc.dram_tensor("ow", [128, 19 * W], F16, kind="ExternalInput").ap()
    bia_ap = nc.dram_tensor("bia", [128, 4], F32, kind="ExternalInput").ap()
    out_ap = nc.dram_tensor("out", [DG, H, W], F32, kind="ExternalOutput").ap()
    scr = nc.dram_tensor("scr", [DG, H + 8, W], F16).ap()


    import contextlib
    with tile.TileContext(nc) as tc, contextlib.ExitStack() as ctx:
        wp = ctx.enter_context(tc.tile_pool(name="wp", bufs=1))
        xp = ctx.enter_context(tc.tile_pool(name="xp", bufs=2))
        hp = ctx.enter_context(tc.tile_pool(name="hp", bufs=2))
        h2p = ctx.enter_context(tc.tile_pool(name="h2p", bufs=3))
        gp = ctx.enter_context(tc.tile_pool(name="gp", bufs=2))
        ps1p = ctx.enter_context(tc.tile_pool(name="ps1p", bufs=2, space="PSUM"))
        ps2p = ctx.enter_context(tc.tile_pool(name="ps2p", bufs=1, space="PSUM"))
        ps3p = ctx.enter_context(tc.tile_pool(name="ps3p", bufs=1, space="PSUM"))

        # ---- weights first, then group-0 x (critical path), then the rest
        wts = wp.tile([128, 544], F16)
        nc.sync.dma_start(out=wts[:], in_=wts_ap[:])
        l1a, l1b = wts[:, 0:128], wts[:, 128:256]
        l2a, l2b = wts[:, 256:384], wts[:, 384:512]
        l3 = wts[:, 512:544]
        xt0 = xp.tile([128, GROUPS[0] * CHUNK_F], F16, tag="x", name="xt0")
        for jj in range(GROUPS[0]):
            nc.sync.dma_start(
                out=xt0[:, jj * CHUNK_F:(jj + 1) * CHUNK_F],
                in_=x_ap[:, :, jj * CHUNK_F:(jj + 1) * CHUNK_F])
        ow = wp.tile([128, 19 * W], F16)
        nc.sync.dma_start(out=ow[:], in_=ow_ap[:])
        offs, wgts = ow[:, 0:18 * W], ow[:, 18 * W:19 * W]
        bia = None
        if not (zb0 and zb1 and zbsim):
            bia = wp.tile([128, 4], F32)
            nc.sync.dma_start(out=bia[:], in_=bia_ap[:])

        # HAM warmup: harmless matmuls on the weight tile while x loads
        ps_w = ps2p.tile([128, 512], F32, tag="ps2", name="ps_warm")
        for _ in range(WARMUP_MM):
            nc.tensor.matmul(ps_w[:, 0:128], wts[:, 0:128], wts[:, 0:128],
                             start=True, stop=True)
        del ps_w

        wgth = wp.tile([128, W], F16)
        nc.scalar.mul(wgth[:], wgts, 0.5)
        wall = wp.tile([128, 18 * W], F16)
        wgth_b = _ap(wgth, 0, [list(wgth.ap[0]), [0, 18], [1, W]])
        nc.vector.tensor_tensor(
            wall[:, :].rearrange("p (s x) -> p s x", s=18),
            offs.rearrange("p (s x) -> p s x", s=18),
            wgth_b, OP.mult)
        # replicate over d so gather muls get contiguous (2x-mode) reads
        wrep = wp.tile([128, 18 * GMAX * W], F16)
        for dd in range(GMAX):
            nc.sync.dma_start(
                out=_ap(wrep, dd * W, [list(wrep.ap[0]), [GMAX * W, 18], [1, W]]),
                in_=_ap(wall, 0, [list(wall.ap[0]), [W, 18], [1, W]]))

        plane0 = 0
        for grp, gn in enumerate(GROUPS):
            planes = list(range(plane0, plane0 + gn))
            plane0 += gn
            gp_rows = 32 * gn

            ps3 = ps3p.tile([gp_rows, 3 * 512], F32, tag="ps3", name="ps3")

            if grp == 0:
                xt = xt0
            else:
                xt = xp.tile([128, gn * CHUNK_F], F16, tag="x")
                nc.sync.dma_start(
                    out=xt[:],
                    in_=x_ap[:, :, planes[0] * CHUNK_F:
                             (planes[0] + gn) * CHUNK_F])

            # ---- conv chain per plane ----
            for j, p in enumerate(planes):
                xv = xt[:, j * CHUNK_F:(j + 1) * CHUNK_F]
                for k, (fo, fn) in enumerate(BLOCKS):
                    ps1 = ps1p.tile([128, 1024], F32, tag="ps1")
                    nc.tensor.matmul(ps1[:, 0:fn], l1a, xv[:, fo:fo + fn],
                                     start=True, stop=True)
                    nc.tensor.matmul(ps1[:, fn:2 * fn], l1b, xv[:, fo:fo + fn],
                                     start=True, stop=True)
                    h1 = hp.tile([128, 1024], F16, tag="h1")
                    eng_v = p in RELU1_DVE_PLANES
                    if zb0:
                        if eng_v:
                            nc.vector.tensor_scalar_max(h1[:, 0:2 * fn],
                                                        ps1[:, 0:2 * fn], 0.0)
                        else:
                            nc.scalar.activation(h1[:, 0:2 * fn], ps1[:, 0:2 * fn],
                                                 AF.Relu)
                    else:
                        for half in (0, 1):
                            sl = slice(half * fn, (half + 1) * fn)
                            bb = bia[:, half:half + 1]
                            if eng_v:
                                nc.vector.tensor_scalar(h1[:, sl], ps1[:, sl],
                                                        bb, 0.0, OP.add, OP.max)
                            else:
                                nc.scalar.activation(h1[:, sl], ps1[:, sl],
                                                     AF.Relu, bias=bb)
                    ps2 = ps2p.tile([128, 512], F32, tag="ps2")
                    nc.tensor.matmul(ps2[:, 0:fn], l2a, h1[:, 0:fn],
                                     start=True, stop=False)
                    nc.tensor.matmul(ps2[:, 0:fn], l2b, h1[:, fn:2 * fn],
                                     start=False, stop=True)
                    h2 = h2p.tile([128, 512], F16, tag="h2")
                    if zb1:
                        nc.scalar.activation(h2[:, 0:fn], ps2[:, 0:fn], AF.Relu)
                    else:
                        nc.scalar.activation(h2[:, 0:fn], ps2[:, 0:fn], AF.Relu,
                                             bias=bia[:, 2:3])
                    nc.tensor.matmul(ps3[32 * j:32 * j + 32, 512 * k:512 * k + fn],
                                     l3, h2[:, 0:fn], start=True, stop=True,
                                     tile_position=(0, 32 * j))

            # ---- evacuate sim (fp16), store to padded scratch, reload ----
            # group 0's evac runs on DVE (ACT is mid-conv); later groups on
            # ACT right after their own relu stream. Megaloads issue from
            # GpSimd (SWDGE) so the SP DMA queue never blocks on them.
            simflat = gp.tile([gp_rows, CHUNK_F], F16, tag="simflat")
            for so, sn in ((0, 1024), (1024, 256)):
                if zbsim:
                    if grp == 0:
                        nc.vector.tensor_copy(simflat[:, so:so + sn],
                                              ps3[:, so:so + sn])
                    else:
                        nc.scalar.copy(simflat[:, so:so + sn], ps3[:, so:so + sn])
                elif grp == 0:
                    nc.vector.tensor_scalar_add(simflat[:, so:so + sn],
                                                ps3[:, so:so + sn],
                                                bia[0:gp_rows, 3:4])
                else:
                    nc.scalar.activation(simflat[:, so:so + sn], ps3[:, so:so + sn],
                                         AF.Identity, bias=bia[0:gp_rows, 3:4])

            yvall = gp.tile([128, 5 * gn * XPAD], F16, tag="yvall")
            for j, p in enumerate(planes):
                sf = simflat[32 * j:32 * j + 16, :]
                nc.sync.dma_start(
                    out=scr[p, 4:132, :],
                    in_=sf.rearrange("c (r x) -> c r x", x=W))
                top = sf[0:1, :].rearrange("o (r x) -> o r x", x=W)
                nc.gpsimd.dma_start(out=scr[p, 0:4, :], in_=top[:, 4:0:-1, :])
                bot = sf[15:16, :].rearrange("o (r x) -> o r x", x=W)
                nc.gpsimd.dma_start(out=scr[p, 132:136, :], in_=bot[:, 6:2:-1, :])
                dst = _ap(yvall, j * XPAD + 4,
                          [list(yvall.ap[0]), [gn * XPAD, 5], [1, W]])
                src_base = scr[p, 0:1, 0:1]
                srcp = bass.AP(tensor=src_base.tensor, offset=src_base.offset,
                               ap=[[W, 128], [2 * W, 5], [1, W]])
                nc.gpsimd.dma_start(out=dst, in_=srcp)
                lp_d = _ap(yvall, j * XPAD,
                           [list(yvall.ap[0]), [gn * XPAD, 5], [1, 4]])
                lp_s = _ap(yvall, j * XPAD + 8,
                           [list(yvall.ap[0]), [gn * XPAD, 5], [-1, 4]])
                nc.vector.tensor_copy(lp_d, lp_s)
                rp_d = _ap(yvall, j * XPAD + 164,
                           [list(yvall.ap[0]), [gn * XPAD, 5], [1, 4]])
                rp_s = _ap(yvall, j * XPAD + 162,
                           [list(yvall.ap[0]), [gn * XPAD, 5], [-1, 4]])
                nc.vector.tensor_copy(rp_d, rp_s)

            # ---- gather: acc = sum_t wrep[t] * shift(sim) (DVE) ----
            # 9 pair-muls write slices of one product tile; then a batched
            # binary add-tree (5 wide ops) folds all 18 products.
            gw = gn * W
            npair = len(TERMS) // 2
            P = gp.tile([128, npair * 2 * gw], F16, tag="gtmp")
            of32 = gp.tile([128, gw], F32, tag="of32")
            for i in range(npair):
                (dyA, dxA, wsA), (dyB, dxB, wsB) = TERMS[2 * i], TERMS[2 * i + 1]
                offA = ((dyA + 4) // 2 * gn) * XPAD + 4 + dxA
                offB = ((dyB + 4) // 2 * gn) * XPAD + 4 + dxB
                srcp = _ap(yvall, offA,
                           [list(yvall.ap[0]), [offB - offA, 2],
                            [XPAD, gn], [1, W]])
                w_b = _ap(wrep, wsA * GMAX * W,
                          [list(wrep.ap[0]), [(wsB - wsA) * GMAX * W, 2],
                           [W, gn], [1, W]])
                dst = P[:, i * 2 * gw:(i + 1) * 2 * gw].rearrange(
                    "p (t d x) -> p t d x", t=2, d=gn)
                nc.vector.tensor_tensor(dst, w_b, srcp, OP.mult)
            # tree over 18 slices of gw: 18 -> 9 -> (4 + carry) -> 2 -> 1
            nc.vector.tensor_tensor(P[:, 0:9 * gw], P[:, 0:9 * gw],
                                    P[:, 9 * gw:18 * gw], OP.add)
            nc.vector.tensor_tensor(P[:, 0:4 * gw], P[:, 0:4 * gw],
                                    P[:, 4 * gw:8 * gw], OP.add)
            nc.vector.tensor_tensor(P[:, 0:2 * gw], P[:, 0:2 * gw],
                                    P[:, 2 * gw:4 * gw], OP.add)
            nc.vector.tensor_tensor(P[:, 0:gw], P[:, 0:gw],
                                    P[:, gw:2 * gw], OP.add)
            nc.vector.tensor_tensor(of32[:, :], P[:, 0:gw],
                                    P[:, 8 * gw:9 * gw], OP.add)
            nc.sync.dma_start(
                out=out_ap[planes[0]:planes[0] + gn].rearrange("d h x -> h d x"),
                in_=of32[:, :].rearrange("p (d x) -> p d x", d=gn))

    nc.compile()
    return nc


def _pack_weights(w0, bn0_scale, bn0_bias, w1, bn1_scale, bn1_bias, w_sim, b_sim):
    w0f = (w0 * bn0_scale[:, None]).astype(np.float32)
    w1f = (w1 * bn1_scale[:, None]).astype(np.float32)
    l1a = np.zeros((128, 128), np.float16)
    l1b = np.zeros((128, 128), np.float16)
    l2a = np.zeros((128, 128), np.float16)
    l2b = np.zeros((128, 128), np.float16)
    l3 = np.zeros((128, 32), np.float16)
    for c in range(NCHUNK):
        s = slice(c * 8, c * 8 + 8)
        l1a[s, s] = w0f[0:8, :].T
        l1b[s, s] = w0f[8:16, :].T
        l2a[s, s] = w1f[:, 0:8].T
        l2b[s, s] = w1f[:, 8:16].T
        l3[s, c] = w_sim[0, :]
    wts = np.hstack([l1a, l1b, l2a, l2b, l3])
    po = np.arange(128) % 8
    bia = np.stack([bn0_bias[po], bn0_bias[po + 8], bn1_bias[po],
                    np.full(128, float(b_sim[0]))], axis=1).astype(np.float32)
    return wts, bia


def prepare(x1, offset, weight, w0, bn0_scale, bn0_bias, w1, bn1_scale, bn1_bias,
            w_sim, b_sim):
    x1 = np.asarray(x1); offset = np.asarray(offset); weight = np.asarray(weight)
    w0 = np.asarray(w0); bn0_scale = np.asarray(bn0_scale)
    bn0_bias = np.asarray(bn0_bias); w1 = np.asarray(w1)
    bn1_scale = np.asarray(bn1_scale); bn1_bias = np.asarray(bn1_bias)
    w_sim = np.asarray(w_sim); b_sim = np.asarray(b_sim)

    wts, bia = _pack_weights(w0, bn0_scale, bn0_bias, w1, bn1_scale, bn1_bias,
                             w_sim, b_sim)
    zb0 = bool(np.all(bn0_bias == 0))
    zb1 = bool(np.all(bn1_bias == 0))
    zbsim = bool(np.all(b_sim == 0))
    nc = _build(zb0, zb1, zbsim)

    in_maps = []
    for core in range(8):
        b, kd = divmod(core, 4)
        ow = np.concatenate([offset[b].transpose(1, 0, 2).reshape(H, 18 * W),
                             weight[b, 0]], axis=1).astype(np.float16)
        xs = x1[b, :, kd * DG:(kd + 1) * DG].astype(np.float16)
        xs = xs.reshape(G, DG, NCHUNK, RPC, W)
        xs = np.ascontiguousarray(xs.transpose(2, 0, 1, 3, 4)).reshape(
            NCHUNK, G, DG * CHUNK_F)
        in_maps.append({"x": xs, "wts": wts, "ow": ow, "bia": bia})
    return nc, in_maps


def kernel(x1, offset, weight, w0, bn0_scale, bn0_bias, w1, bn1_scale, bn1_bias,
           w_sim, b_sim):
    nc, in_maps = prepare(x1, offset, weight, w0, bn0_scale, bn0_bias, w1,
                          bn1_scale, bn1_bias, w_sim, b_sim)
    res = run_bass_kernel_spmd(nc, in_maps, list(range(8)))
    out = np.empty((B, D, H, W), np.float32)
    for core in range(8):
        b, kd = divmod(core, 4)
        out[b, kd * DG:(kd + 1) * DG] = res.results[core]["out"]
    return out
